# revision 52
# baseline (speedup 1.0000x reference)
"""ChemProp message-to-node + MLP kernel for 8 TRN2 NeuronCores.

Strategy (no collectives needed):
  - Host assigns nodes to cores by global degree rank, round-robin, so all
    cores see near-identical degree sequences.  Within a core, nodes are
    dealt round-robin into <=512-node groups (one PSUM window each).
  - Edge features stream in fp8 (e3m4) with host-side error-feedback
    quantization: each node's edges are rounded so quantization residuals
    carry into the next edge, making the on-device segment-sum nearly
    exact despite the 1-byte stream.
  - Segment-sum as true matmuls: edges are packed two-per-partition
    (512 B contiguous per partition keeps DMA at full bandwidth).  For
    each 256-edge chunk, 4 small matmuls (2 feature ptiles x even/odd
    slot) contract 128 edges at a time against a 0/1 aggregation matrix
    A streamed from HBM, accumulating the group's [128, w] message tile
    in PSUM.  Chunk boundaries are shared across cores via a max-degree
    ("ub") slot layout, so one Bass graph serves all 8 cores.
  - MLP runs in bf16 with f32 PSUM accumulation, feature-major, exactly
    as the reference: hidden^T = relu(W1^T @ [r; msg]^T), out = W2^T @ h.
  - Per-core output is returned feature-major bf16; host transposes,
    un-permutes, casts to f32 and concatenates.
"""

import numpy as np
import ml_dtypes

import concourse.bacc as bacc
import concourse.mybir as mybir
import concourse.tile as tile
from concourse.bass_utils import run_bass_kernel_spmd

NC = 8            # cores
# group caps: small first group -> fast pipeline fill; small last groups ->
# short drain; middle groups fill one PSUM f32 bank (512 cols)
CAPS = (256,) + (512,) * 10 + (352, 234, 162, 126)
SLOTS = 256       # edge slots per chunk (2 per partition)
STREAM_BUFS = 3
MSG_BUFS = 2
PSUM_MSG_BUFS = 2
HID_BUFS = 2
RT_BATCH = 4      # groups per rT load strip
OUT_BATCH = 2     # groups per out store strip

BF16 = mybir.dt.bfloat16
F32 = mybir.dt.float32
FP8 = mybir.dt.float8e3
NP_BF16 = ml_dtypes.bfloat16
NP_FP8 = ml_dtypes.float8_e3m4


# ----------------------------------------------------------------- host side
def _quantize_h(h, dst, n_nodes):
    """fp8 e3m4 with per-node error feedback: quantization residual of each
    edge is carried into the node's next edge, so the device-side sum of the
    quantized values tracks the exact sum to ~half an ulp."""
    deg = np.bincount(dst, minlength=n_nodes)
    order = np.argsort(dst, kind="stable")
    starts = np.zeros(n_nodes + 1, dtype=np.int64)
    np.cumsum(deg, out=starts[1:])
    hq = np.zeros(h.shape, dtype=NP_FP8)
    carry = np.zeros((n_nodes, h.shape[1]), dtype=np.float32)
    for k in range(int(deg.max())):
        sel = np.nonzero(deg > k)[0]
        eids = order[starts[sel] + k]
        val = h[eids].astype(np.float32) + carry[sel]
        q = val.astype(NP_FP8)
        carry[sel] = val - q.astype(np.float32)
        hq[eids] = q
    return hq, deg, order, starts


def _preprocess(deg_flat):
    """Node->core/group assignment + shared chunk schedule."""
    n_nodes = deg_flat.shape[0]
    npc = n_nodes // NC
    caps = list(CAPS)
    assert sum(caps) == npc and max(caps) <= 512
    grp_lo = np.concatenate([[0], np.cumsum(caps)]).astype(np.int64)
    ngrp = len(caps)

    # global degree rank, round-robin over cores, then round-robin over
    # groups within the core (fill order = degree-desc within group)
    rank = np.argsort(-deg_flat, kind="stable")
    node_ids = np.zeros((NC, npc), dtype=np.int64)
    for c in range(NC):
        ids_q = rank[c::NC]
        fill = [0] * ngrp
        g = 0
        for q in range(npc):
            while fill[g] == caps[g]:
                g = (g + 1) % ngrp
            node_ids[c, int(grp_lo[g]) + fill[g]] = ids_q[q]
            fill[g] += 1
            g = (g + 1) % ngrp
    deg_sorted = deg_flat[node_ids]                     # [NC, npc]

    # shared slot layout: node at (g, i) owns slots [cum_ub[i], cum_ub[i+1])
    # of group g, where deg_ub = max degree over cores at that position.
    deg_ub = deg_sorted.max(axis=0)                     # [npc]
    chunk_base = [0] * ngrp            # first global chunk of each group
    nchunks = [0] * ngrp
    spans = [None] * ngrp              # per chunk: (pos_lo, pos_hi)
    cum_ub_g = [None] * ngrp
    tot_chunks = 0
    for g in range(ngrp):
        lo, hi = int(grp_lo[g]), int(grp_lo[g + 1])
        ub = deg_ub[lo:hi]
        cum = np.zeros(hi - lo + 1, dtype=np.int64)
        np.cumsum(ub, out=cum[1:])
        cum_ub_g[g] = cum
        nch = max(int(-(-cum[-1] // SLOTS)), 1)
        chunk_base[g] = tot_chunks
        nchunks[g] = nch
        tot_chunks += nch
        sp = []
        for ch in range(nch):
            s0, s1 = ch * SLOTS, (ch + 1) * SLOTS
            # nodes whose slot window intersects [s0, s1); zero-degree nodes
            # fall into the chunk their cum position lands in
            p_lo = int(np.searchsorted(cum[1:], s0, side="right"))
            p_hi = int(np.searchsorted(cum[:-1], s1, side="left"))
            p_hi = max(p_hi, p_lo + 1)
            sp.append((p_lo, min(p_hi, hi - lo)))
        # every position must be covered by >=1 span so its PSUM message
        # column is written (zero-degree tail nodes otherwise fall through)
        sp[0] = (0, sp[0][1])
        sp[-1] = (sp[-1][0], hi - lo)
        for i in range(len(sp) - 1):
            assert sp[i + 1][0] <= sp[i][1]
        spans[g] = sp

    # A-matrix column layout: per group, per chunk: [even: W][odd: W]
    a_base = [0] * ngrp
    a_cols_g = [None] * ngrp
    a_tot = 0
    for g in range(ngrp):
        offs = []
        off = 0
        for (p_lo, p_hi) in spans[g]:
            offs.append(off)
            off += 2 * (p_hi - p_lo)
        a_base[g] = a_tot
        a_cols_g[g] = offs + [off]
        a_tot += off

    # merged per-group DMA block: [A acols | stream nch*512] fp8
    # (512 = 2 edges x 256 features per partition per chunk)
    m_base = [0] * ngrp
    m_tot = 0
    for g in range(ngrp):
        m_base[g] = m_tot
        m_tot += a_cols_g[g][-1] + nchunks[g] * 512

    return {
        "npc": npc, "ngrp": ngrp, "caps": caps, "grp_lo": grp_lo,
        "node_ids": node_ids, "deg_sorted": deg_sorted,
        "cum_ub_g": cum_ub_g, "chunk_base": chunk_base, "nchunks": nchunks,
        "spans": spans, "tot_chunks": tot_chunks,
        "a_base": a_base, "a_cols_g": a_cols_g, "a_tot": a_tot,
        "m_base": m_base, "m_tot": m_tot,
    }


def _build_streams(hq, r, lay, order, starts):
    """Materialize per-core device arrays: edge stream, A matrices, rT."""
    Fdim = hq.shape[1]
    npc, ngrp = lay["npc"], lay["ngrp"]
    tot_chunks, a_tot = lay["tot_chunks"], lay["a_tot"]
    grp_lo = lay["grp_lo"]
    fp = Fdim // 128

    hs_l, A_l, rT_l = [], [], []
    for c in range(NC):
        # edge -> slot assignment (vectorized per group)
        stream = np.zeros((tot_chunks * 128 * 2, Fdim), dtype=NP_FP8)
        A = np.zeros((128, a_tot), dtype=NP_FP8)
        for g in range(ngrp):
            lo, hi = int(grp_lo[g]), int(grp_lo[g + 1])
            nodes = lay["node_ids"][c, lo:hi]
            degs = lay["deg_sorted"][c, lo:hi]
            cum = lay["cum_ub_g"][g]
            E = int(degs.sum())
            if E == 0:
                continue
            loc = np.arange(E, dtype=np.int64) - np.repeat(
                np.concatenate([[0], np.cumsum(degs)[:-1]]), degs)
            slots = np.repeat(cum[:-1], degs) + loc        # slot in group
            eids = order[np.repeat(starts[nodes], degs) + loc]
            pos = np.repeat(np.arange(hi - lo, dtype=np.int64), degs)

            ch = slots // SLOTS
            within = slots % SLOTS
            part = within // 2
            parity = within % 2
            # stream row index: ((chunk_global*128 + part)*2 + parity)
            row = ((lay["chunk_base"][g] + ch) * 128 + part) * 2 + parity
            stream[row] = hq[eids]
            # A entry: col = a_base + chunk_off + parity*W + (pos - p_lo)
            offs = np.asarray(lay["a_cols_g"][g][:-1], dtype=np.int64)
            p_los = np.asarray([s[0] for s in lay["spans"][g]], dtype=np.int64)
            p_his = np.asarray([s[1] for s in lay["spans"][g]], dtype=np.int64)
            W = p_his - p_los
            col = lay["a_base"][g] + offs[ch] + parity * W[ch] + (pos - p_los[ch])
            assert (pos >= p_los[ch]).all() and (pos < p_his[ch]).all()
            A[part, col] = np.float32(1.0)

        # [tot_chunks, 128, 2, F] -> [128, tot_chunks * 2F] partition-major,
        # then interleave each group's A block in front of its stream block
        st = stream.reshape(tot_chunks, 128, 2 * Fdim)
        sm = np.ascontiguousarray(st.transpose(1, 0, 2)).reshape(128, -1)
        merged = np.zeros((128, lay["m_tot"]), dtype=NP_FP8)
        for g in range(ngrp):
            mb = lay["m_base"][g]
            ab = lay["a_base"][g]
            acols = lay["a_cols_g"][g][-1]
            cb = lay["chunk_base"][g]
            nch = lay["nchunks"][g]
            merged[:, mb:mb + acols] = A[:, ab:ab + acols]
            merged[:, mb + acols:mb + acols + nch * 512] = \
                sm[:, cb * 512:(cb + nch) * 512]
        hs_l.append(merged)
        rc = r[lay["node_ids"][c]].astype(NP_FP8)
        rT_l.append(np.ascontiguousarray(rc.T).reshape(fp, 128, npc))
    return hs_l, rT_l


# --------------------------------------------------------------- device side
def _build_graph(lay, Fdim, H, Fout):
    npc, ngrp = lay["npc"], lay["ngrp"]
    fp = Fdim // 128          # 2 feature ptiles
    kt_n = (2 * Fdim) // 128  # 4 k-chunks for W1
    ht_n = H // 128           # 4 hidden ptiles
    ot_n = Fout // 128        # 2 output ptiles

    nc = bacc.Bacc(None, target_bir_lowering=False)
    hs_p = nc.declare_dram_parameter("hs", [128, lay["m_tot"]], FP8, isOutput=False)
    rT_p = nc.declare_dram_parameter("rT", [fp, 128, npc], FP8, isOutput=False)
    # W1 and W2 packed partition-major into one blob -> single DMA issue
    wb_cols = kt_n * H + ht_n * Fout
    wb_p = nc.declare_dram_parameter("Wb", [128, wb_cols], BF16, isOutput=False)
    out_p = nc.declare_dram_parameter("out", [ot_n, 128, npc], BF16, isOutput=True)

    with tile.TileContext(nc) as tc:
        with (
            tc.tile_pool(name="const", bufs=1) as const_pool,
            tc.tile_pool(name="stream", bufs=STREAM_BUFS) as stream_pool,
            tc.tile_pool(name="msgp", bufs=1, space="PSUM") as msg_psum_pool,
            tc.tile_pool(name="msgb", bufs=MSG_BUFS) as msg_pool,
            tc.tile_pool(name="rb", bufs=2) as r_pool,
            tc.tile_pool(name="mlp1p", bufs=3, space="PSUM") as mlp1_psum_pool,
            tc.tile_pool(name="mlp2p", bufs=1, space="PSUM") as mlp2_psum_pool,
            tc.tile_pool(name="hid", bufs=HID_BUFS) as hid_pool,
            tc.tile_pool(name="osb", bufs=3) as out_pool,
        ):
            # weights resident in SBUF; one DMA on the SP queue ahead of the
            # first edge-stream DMA so MLP(g0) never stalls on them
            wb_t = const_pool.tile([128, wb_cols], BF16, tag="wb")
            nc.sync.dma_start(out=wb_t[:], in_=wb_p[:])

            def w1_sl(k, ht):
                c = k * H + ht * 128
                return wb_t[:, c:c + 128]

            def w2_sl(k, ot):
                c = kt_n * H + k * Fout + ot * 128
                return wb_t[:, c:c + 128]

            # out-store batches: OUT_BATCH groups each, but the last two
            # groups flush individually so the final store doesn't wait on
            # two MLPs
            batch_of = {}
            batches = []
            g = 0
            while g < ngrp:
                n = 1 if g >= ngrp - 2 else min(OUT_BATCH, ngrp - 2 - g)
                batches.append((g, g + n))
                for x in range(g, g + n):
                    batch_of[x] = len(batches) - 1
                g += n
            ob_state = {}

            def emit_mlp(pend):
                gi, lo, w_g = pend["gi"], pend["lo"], pend["w_g"]
                cat = pend["rb"] + pend["msgb"]  # k-chunks match W1 rows
                b_first, b_last = batches[batch_of[gi]]
                hid = []
                for ht in range(ht_n):
                    ps = mlp1_psum_pool.tile([128, w_g], F32, space="PSUM",
                                             tag="mlp1")
                    for k in range(kt_n):
                        nc.tensor.matmul(
                            out=ps[:],
                            lhsT=w1_sl(k, ht),
                            rhs=cat[k][:],
                            start=(k == 0), stop=(k == kt_n - 1))
                    hb = hid_pool.tile([128, w_g], BF16, tag=f"h{ht}")
                    if ht % 2 == 0:
                        nc.scalar.activation(
                            hb[:], ps[:], mybir.ActivationFunctionType.Relu)
                    else:
                        nc.vector.tensor_scalar_max(hb[:], ps[:], 0.0)
                    hid.append(hb)
                # k-major so the last-relu'd hidden tile is consumed LAST,
                # with both output tiles' earlier k-chunks runnable before it
                ps2 = []
                for ot in range(ot_n):
                    ps2_t = mlp2_psum_pool.tile([128, w_g], F32, space="PSUM",
                                                tag=f"mlp2_{ot}")
                    ps2.append(ps2_t)
                for k in range(ht_n):
                    for ot in range(ot_n):
                        nc.tensor.matmul(
                            out=ps2[ot][:],
                            lhsT=w2_sl(k, ot),
                            rhs=hid[k][:],
                            start=(k == 0), stop=(k == ht_n - 1))

                if gi == b_first:
                    ob_state["lo"] = lo
                    ob_state["hi"] = int(lay["grp_lo"][b_last])
                    strips = []
                    for o in range(ot_n):
                        ob_t = out_pool.tile(
                            [128, ob_state["hi"] - ob_state["lo"]],
                            BF16, tag=f"o{o}")
                        strips.append(ob_t)
                    ob_state["strips"] = strips
                ob_lo = ob_state["lo"]
                for ot in range(ot_n):
                    nc.vector.tensor_copy(
                        out=ob_state["strips"][ot][:, lo - ob_lo:
                                                   lo - ob_lo + w_g],
                        in_=ps2[ot][:])
                    if gi == b_last - 1:
                        q = nc.scalar if ot == 0 else nc.sync
                        q.dma_start(
                            out=out_p[ot, :, ob_lo:
                                      ob_lo + ob_state["strips"][ot].shape[1]],
                            in_=ob_state["strips"][ot][:])

            pend = None
            for gi in range(ngrp):
                lo = int(lay["grp_lo"][gi])
                w_g = int(lay["grp_lo"][gi + 1]) - lo
                nch = lay["nchunks"][gi]
                acols = lay["a_cols_g"][gi][-1]
                mb = lay["m_base"][gi]

                # ---- merged [A | stream] DMA for this group (SP queue)
                st = stream_pool.tile([128, acols + nch * 512], FP8, tag="hs")
                nc.sync.dma_start(
                    out=st[:], in_=hs_p[:, mb:mb + acols + nch * 512])

                # ---- rT strip (bf16, feature-major), RT_BATCH groups
                if gi % RT_BATCH == 0:
                    b_lo = lo
                    b_hi = int(lay["grp_lo"][min(gi + RT_BATCH, ngrp)])
                    rb_strip = []
                    for p in range(fp):
                        t = r_pool.tile([128, b_hi - b_lo], FP8, tag=f"rb{p}")
                        nc.gpsimd.dma_start(out=t[:], in_=rT_p[p, :, b_lo:b_hi])
                        rb_strip.append(t)
                    rb_base = b_lo
                rb = [t[:, lo - rb_base:lo - rb_base + w_g] for t in rb_strip]

                # ---- segment-sum: per chunk, 4 matmuls (ptile x parity)
                # contract 128 edges at a time against the 0/1 A matrix
                ps_msg = []
                for p in range(fp):
                    mp_t = msg_psum_pool.tile([128, w_g], F32, space="PSUM",
                                              tag=f"mp{p}")
                    ps_msg.append(mp_t)
                offs = lay["a_cols_g"][gi]
                spans = lay["spans"][gi]
                for ch in range(nch):
                    p_lo, p_hi = spans[ch]
                    Wc = p_hi - p_lo
                    for parity in range(2):
                        a_sl = st[:, offs[ch] + parity * Wc:
                                  offs[ch] + (parity + 1) * Wc]
                        for p in range(fp):
                            c0 = acols + ch * 512 + parity * Fdim + p * 128
                            nc.tensor.matmul(
                                out=ps_msg[p][:, p_lo:p_hi],
                                lhsT=st[:, c0:c0 + 128],
                                rhs=a_sl,
                                start=(ch == 0 and parity == 0),
                                stop=(ch == nch - 1 and parity == 1),
                                skip_group_check=True,
                            )
                msgb = []
                for p in range(fp):
                    mb_t = msg_pool.tile([128, w_g], BF16, tag=f"mb{p}")
                    if p == 0:
                        nc.vector.tensor_copy(out=mb_t[:], in_=ps_msg[p][:])
                    else:
                        nc.scalar.activation(
                            mb_t[:], ps_msg[p][:],
                            mybir.ActivationFunctionType.Copy)
                    msgb.append(mb_t)

                # software pipeline: MLP of the previous group is emitted
                # AFTER this group's segment-sum so the PE never idles
                # waiting for message copies; the last two groups drop to
                # depth 0 so their MLPs overlap the tail streams
                emit_mlp({"gi": gi, "lo": lo, "w_g": w_g, "rb": rb,
                          "msgb": msgb})

    nc.finalize()
    return nc


# ----------------------------------------------------------------- interface
def prepare(r, h, nbrs, W1, W2):
    r = np.asarray(r, dtype=np.float32)
    h = np.asarray(h, dtype=np.float32)
    nbrs = np.asarray(nbrs)
    W1 = np.asarray(W1, dtype=np.float32)
    W2 = np.asarray(W2, dtype=np.float32)

    n_nodes, Fdim = r.shape
    H = W1.shape[1]
    Fout = W2.shape[1]

    dst = nbrs[:, 0].astype(np.int64)
    hq, deg, order, starts = _quantize_h(h, dst, n_nodes)
    lay = _preprocess(deg)
    hs_l, rT_l = _build_streams(hq, r, lay, order, starts)
    kt_n = (2 * Fdim) // 128
    ht_n = H // 128
    wb = np.zeros((128, kt_n * H + ht_n * Fout), dtype=NP_BF16)
    for k in range(kt_n):
        wb[:, k * H:(k + 1) * H] = W1[k * 128:(k + 1) * 128, :].astype(NP_BF16)
    for k in range(ht_n):
        wb[:, kt_n * H + k * Fout:kt_n * H + (k + 1) * Fout] = \
            W2[k * 128:(k + 1) * 128, :].astype(NP_BF16)

    nc = _build_graph(lay, Fdim, H, Fout)
    in_maps = [
        {"hs": hs_l[c], "rT": rT_l[c], "Wb": wb}
        for c in range(NC)
    ]
    return {"nc": nc, "in_maps": in_maps, "lay": lay,
            "n_nodes": n_nodes, "Fout": Fout}


def assemble(prep, results):
    lay = prep["lay"]
    n_nodes, Fout = prep["n_nodes"], prep["Fout"]
    npc = lay["npc"]
    out = np.zeros((n_nodes, Fout), dtype=np.float32)
    for c in range(NC):
        o = np.asarray(results[c]["out"]).reshape(Fout, npc)
        out[lay["node_ids"][c]] = o.T.astype(np.float32)
    return out


def kernel(r, h, nbrs, W1, W2):
    prep = prepare(r, h, nbrs, W1, W2)
    res = run_bass_kernel_spmd(prep["nc"], prep["in_maps"],
                               core_ids=list(range(NC)))
    return assemble(prep, res.results)


# revision 55
# speedup vs baseline: 1.0663x; 1.0663x over previous
"""ChemProp message-to-node + MLP kernel for 8 TRN2 NeuronCores.

Strategy (no collectives needed):
  - Host assigns nodes to cores by global degree rank, round-robin, so all
    cores see near-identical degree sequences.  Within a core, nodes are
    dealt round-robin into <=512-node groups (one PSUM window each).
  - Edge features stream in fp8 (e3m4) with host-side error-feedback
    quantization: each node's edges are rounded so quantization residuals
    carry into the next edge, making the on-device segment-sum nearly
    exact despite the 1-byte stream.
  - Segment-sum as true matmuls: edges are packed two-per-partition
    (512 B contiguous per partition keeps DMA at full bandwidth).  For
    each 256-edge chunk, 4 small matmuls (2 feature ptiles x even/odd
    slot) contract 128 edges at a time against a 0/1 aggregation matrix
    A streamed from HBM, accumulating the group's [128, w] message tile
    in PSUM.  Chunk boundaries are shared across cores via a max-degree
    ("ub") slot layout, so one Bass graph serves all 8 cores.
  - MLP runs in bf16 with f32 PSUM accumulation, feature-major, exactly
    as the reference: hidden^T = relu(W1^T @ [r; msg]^T), out = W2^T @ h.
  - Per-core output is returned feature-major bf16; host transposes,
    un-permutes, casts to f32 and concatenates.
"""

import numpy as np
import ml_dtypes

import concourse.bacc as bacc
import concourse.mybir as mybir
import concourse.tile as tile
from concourse.bass_utils import run_bass_kernel_spmd

NC = 8            # cores
# group caps: small first group -> fast pipeline fill; small last groups ->
# short drain; middle groups fill one PSUM f32 bank (512 cols)
CAPS = (256,) + (512,) * 10 + (352, 234, 162, 126)
SLOTS = 256       # edge slots per chunk (2 per partition)
STREAM_BUFS = 3
MSG_BUFS = 2
PSUM_MSG_BUFS = 2
HID_BUFS = 2
RT_BATCH = 4      # groups per rT load strip
OUT_BATCH = 2     # groups per out store strip

BF16 = mybir.dt.bfloat16
F32 = mybir.dt.float32
FP8 = mybir.dt.float8e3
NP_BF16 = ml_dtypes.bfloat16
NP_FP8 = ml_dtypes.float8_e3m4


# ----------------------------------------------------------------- host side
def _quantize_h(h, dst, n_nodes):
    """fp8 e3m4 with per-node error feedback: quantization residual of each
    edge is carried into the node's next edge, so the device-side sum of the
    quantized values tracks the exact sum to ~half an ulp."""
    deg = np.bincount(dst, minlength=n_nodes)
    order = np.argsort(dst, kind="stable")
    starts = np.zeros(n_nodes + 1, dtype=np.int64)
    np.cumsum(deg, out=starts[1:])
    hq = np.zeros(h.shape, dtype=NP_FP8)
    carry = np.zeros((n_nodes, h.shape[1]), dtype=np.float32)
    for k in range(int(deg.max())):
        sel = np.nonzero(deg > k)[0]
        eids = order[starts[sel] + k]
        val = h[eids].astype(np.float32) + carry[sel]
        q = val.astype(NP_FP8)
        carry[sel] = val - q.astype(np.float32)
        hq[eids] = q
    return hq, deg, order, starts


def _preprocess(deg_flat):
    """Node->core/group assignment + shared chunk schedule."""
    n_nodes = deg_flat.shape[0]
    npc = n_nodes // NC
    caps = list(CAPS)
    assert sum(caps) == npc and max(caps) <= 512
    grp_lo = np.concatenate([[0], np.cumsum(caps)]).astype(np.int64)
    ngrp = len(caps)

    # global degree rank, round-robin over cores, then round-robin over
    # groups within the core (fill order = degree-desc within group)
    rank = np.argsort(-deg_flat, kind="stable")
    node_ids = np.zeros((NC, npc), dtype=np.int64)
    for c in range(NC):
        ids_q = rank[c::NC]
        fill = [0] * ngrp
        g = 0
        for q in range(npc):
            while fill[g] == caps[g]:
                g = (g + 1) % ngrp
            node_ids[c, int(grp_lo[g]) + fill[g]] = ids_q[q]
            fill[g] += 1
            g = (g + 1) % ngrp
    deg_sorted = deg_flat[node_ids]                     # [NC, npc]

    # shared slot layout: node at (g, i) owns slots [cum_ub[i], cum_ub[i+1])
    # of group g, where deg_ub = max degree over cores at that position.
    deg_ub = deg_sorted.max(axis=0)                     # [npc]
    chunk_base = [0] * ngrp            # first global chunk of each group
    nchunks = [0] * ngrp
    spans = [None] * ngrp              # per chunk: (pos_lo, pos_hi)
    cum_ub_g = [None] * ngrp
    tot_chunks = 0
    for g in range(ngrp):
        lo, hi = int(grp_lo[g]), int(grp_lo[g + 1])
        ub = deg_ub[lo:hi]
        cum = np.zeros(hi - lo + 1, dtype=np.int64)
        np.cumsum(ub, out=cum[1:])
        cum_ub_g[g] = cum
        nch = max(int(-(-cum[-1] // SLOTS)), 1)
        chunk_base[g] = tot_chunks
        nchunks[g] = nch
        tot_chunks += nch
        sp = []
        for ch in range(nch):
            s0, s1 = ch * SLOTS, (ch + 1) * SLOTS
            # nodes whose slot window intersects [s0, s1); zero-degree nodes
            # fall into the chunk their cum position lands in
            p_lo = int(np.searchsorted(cum[1:], s0, side="right"))
            p_hi = int(np.searchsorted(cum[:-1], s1, side="left"))
            p_hi = max(p_hi, p_lo + 1)
            sp.append((p_lo, min(p_hi, hi - lo)))
        # every position must be covered by >=1 span so its PSUM message
        # column is written (zero-degree tail nodes otherwise fall through)
        sp[0] = (0, sp[0][1])
        sp[-1] = (sp[-1][0], hi - lo)
        for i in range(len(sp) - 1):
            assert sp[i + 1][0] <= sp[i][1]
        spans[g] = sp

    # A-matrix column layout: per group, per chunk: [even: W][odd: W]
    a_base = [0] * ngrp
    a_cols_g = [None] * ngrp
    a_tot = 0
    for g in range(ngrp):
        offs = []
        off = 0
        for (p_lo, p_hi) in spans[g]:
            offs.append(off)
            off += 2 * (p_hi - p_lo)
        a_base[g] = a_tot
        a_cols_g[g] = offs + [off]
        a_tot += off

    # merged per-group DMA block: [A acols | stream nch*512] fp8
    # (512 = 2 edges x 256 features per partition per chunk)
    m_base = [0] * ngrp
    m_tot = 0
    for g in range(ngrp):
        m_base[g] = m_tot
        m_tot += a_cols_g[g][-1] + nchunks[g] * 512

    return {
        "npc": npc, "ngrp": ngrp, "caps": caps, "grp_lo": grp_lo,
        "node_ids": node_ids, "deg_sorted": deg_sorted,
        "cum_ub_g": cum_ub_g, "chunk_base": chunk_base, "nchunks": nchunks,
        "spans": spans, "tot_chunks": tot_chunks,
        "a_base": a_base, "a_cols_g": a_cols_g, "a_tot": a_tot,
        "m_base": m_base, "m_tot": m_tot,
    }


def _build_streams(hq, r, lay, order, starts):
    """Materialize per-core device arrays: edge stream, A matrices, rT."""
    Fdim = hq.shape[1]
    npc, ngrp = lay["npc"], lay["ngrp"]
    tot_chunks, a_tot = lay["tot_chunks"], lay["a_tot"]
    grp_lo = lay["grp_lo"]
    fp = Fdim // 128

    hs_l, A_l, rT_l = [], [], []
    for c in range(NC):
        # edge -> slot assignment (vectorized per group)
        stream = np.zeros((tot_chunks * 128 * 2, Fdim), dtype=NP_FP8)
        A = np.zeros((128, a_tot), dtype=NP_FP8)
        for g in range(ngrp):
            lo, hi = int(grp_lo[g]), int(grp_lo[g + 1])
            nodes = lay["node_ids"][c, lo:hi]
            degs = lay["deg_sorted"][c, lo:hi]
            cum = lay["cum_ub_g"][g]
            E = int(degs.sum())
            if E == 0:
                continue
            loc = np.arange(E, dtype=np.int64) - np.repeat(
                np.concatenate([[0], np.cumsum(degs)[:-1]]), degs)
            slots = np.repeat(cum[:-1], degs) + loc        # slot in group
            eids = order[np.repeat(starts[nodes], degs) + loc]
            pos = np.repeat(np.arange(hi - lo, dtype=np.int64), degs)

            ch = slots // SLOTS
            within = slots % SLOTS
            part = within // 2
            parity = within % 2
            # stream row index: ((chunk_global*128 + part)*2 + parity)
            row = ((lay["chunk_base"][g] + ch) * 128 + part) * 2 + parity
            stream[row] = hq[eids]
            # A entry: col = a_base + chunk_off + parity*W + (pos - p_lo)
            offs = np.asarray(lay["a_cols_g"][g][:-1], dtype=np.int64)
            p_los = np.asarray([s[0] for s in lay["spans"][g]], dtype=np.int64)
            p_his = np.asarray([s[1] for s in lay["spans"][g]], dtype=np.int64)
            W = p_his - p_los
            col = lay["a_base"][g] + offs[ch] + parity * W[ch] + (pos - p_los[ch])
            assert (pos >= p_los[ch]).all() and (pos < p_his[ch]).all()
            A[part, col] = np.float32(1.0)

        # [tot_chunks, 128, 2, F] -> [128, tot_chunks * 2F] partition-major,
        # then interleave each group's A block in front of its stream block
        st = stream.reshape(tot_chunks, 128, 2 * Fdim)
        sm = np.ascontiguousarray(st.transpose(1, 0, 2)).reshape(128, -1)
        merged = np.zeros((128, lay["m_tot"]), dtype=NP_FP8)
        for g in range(ngrp):
            mb = lay["m_base"][g]
            ab = lay["a_base"][g]
            acols = lay["a_cols_g"][g][-1]
            cb = lay["chunk_base"][g]
            nch = lay["nchunks"][g]
            merged[:, mb:mb + acols] = A[:, ab:ab + acols]
            merged[:, mb + acols:mb + acols + nch * 512] = \
                sm[:, cb * 512:(cb + nch) * 512]
        hs_l.append(merged)
        # rT strips: per RT_BATCH block, both feature ptiles side by side so
        # one DMA fetches the whole strip: cols [2*b_lo:2*b_hi] =
        # [p0 cols b_lo..b_hi | p1 cols b_lo..b_hi]
        rc = r[lay["node_ids"][c]].astype(NP_FP8)
        rt = np.ascontiguousarray(rc.T).reshape(fp, 128, npc)
        rti = np.zeros((128, fp * npc), dtype=NP_FP8)
        bs = [int(x) for x in lay["grp_lo"][::RT_BATCH]] + [npc]
        for b_lo, b_hi in zip(bs[:-1], bs[1:]):
            w = b_hi - b_lo
            for p in range(fp):
                rti[:, fp * b_lo + p * w:fp * b_lo + (p + 1) * w] = \
                    rt[p, :, b_lo:b_hi]
        rT_l.append(rti)
    return hs_l, rT_l


# --------------------------------------------------------------- device side
def _build_graph(lay, Fdim, H, Fout):
    npc, ngrp = lay["npc"], lay["ngrp"]
    fp = Fdim // 128          # 2 feature ptiles
    kt_n = (2 * Fdim) // 128  # 4 k-chunks for W1
    ht_n = H // 128           # 4 hidden ptiles
    ot_n = Fout // 128        # 2 output ptiles

    nc = bacc.Bacc(None, target_bir_lowering=False)
    hs_p = nc.declare_dram_parameter("hs", [128, lay["m_tot"]], FP8, isOutput=False)
    rT_p = nc.declare_dram_parameter("rT", [128, fp * npc], FP8, isOutput=False)
    # W1 and W2 packed partition-major into one blob -> single DMA issue
    wb_cols = kt_n * H + ht_n * Fout
    wb_p = nc.declare_dram_parameter("Wb", [128, wb_cols], BF16, isOutput=False)
    out_p = nc.declare_dram_parameter("out", [ot_n, 128, npc], BF16, isOutput=True)

    with tile.TileContext(nc) as tc:
        with (
            tc.tile_pool(name="const", bufs=1) as const_pool,
            tc.tile_pool(name="stream", bufs=STREAM_BUFS) as stream_pool,
            tc.tile_pool(name="msgp", bufs=1, space="PSUM") as msg_psum_pool,
            tc.tile_pool(name="msgb", bufs=MSG_BUFS) as msg_pool,
            tc.tile_pool(name="rb", bufs=2) as r_pool,
            tc.tile_pool(name="mlp1p", bufs=3, space="PSUM") as mlp1_psum_pool,
            tc.tile_pool(name="mlp2p", bufs=1, space="PSUM") as mlp2_psum_pool,
            tc.tile_pool(name="hid", bufs=HID_BUFS) as hid_pool,
            tc.tile_pool(name="osb", bufs=3) as out_pool,
        ):
            # weights resident in SBUF; one DMA on the SP queue ahead of the
            # first edge-stream DMA so MLP(g0) never stalls on them
            wb_t = const_pool.tile([128, wb_cols], BF16, tag="wb")
            nc.sync.dma_start(out=wb_t[:], in_=wb_p[:])

            def w1_sl(k, ht):
                c = k * H + ht * 128
                return wb_t[:, c:c + 128]

            def w2_sl(k, ot):
                c = kt_n * H + k * Fout + ot * 128
                return wb_t[:, c:c + 128]

            # out-store batches: OUT_BATCH groups each, but the last two
            # groups flush individually so the final store doesn't wait on
            # two MLPs
            batch_of = {}
            batches = []
            g = 0
            while g < ngrp:
                n = 1 if g >= ngrp - 2 else min(OUT_BATCH, ngrp - 2 - g)
                batches.append((g, g + n))
                for x in range(g, g + n):
                    batch_of[x] = len(batches) - 1
                g += n
            ob_state = {}

            def emit_mlp(pend):
                gi, lo, w_g = pend["gi"], pend["lo"], pend["w_g"]
                cat = pend["rb"] + pend["msgb"]  # k-chunks match W1 rows
                b_first, b_last = batches[batch_of[gi]]
                hid = []
                for ht in range(ht_n):
                    ps = mlp1_psum_pool.tile([128, w_g], F32, space="PSUM",
                                             tag="mlp1")
                    for k in range(kt_n):
                        nc.tensor.matmul(
                            out=ps[:],
                            lhsT=w1_sl(k, ht),
                            rhs=cat[k][:],
                            start=(k == 0), stop=(k == kt_n - 1))
                    hb = hid_pool.tile([128, w_g], BF16, tag=f"h{ht}")
                    if ht % 2 == 0:
                        nc.scalar.activation(
                            hb[:], ps[:], mybir.ActivationFunctionType.Relu)
                    else:
                        nc.vector.tensor_scalar_max(hb[:], ps[:], 0.0)
                    hid.append(hb)
                # k-major so the last-relu'd hidden tile is consumed LAST,
                # with both output tiles' earlier k-chunks runnable before it
                ps2 = []
                for ot in range(ot_n):
                    ps2_t = mlp2_psum_pool.tile([128, w_g], F32, space="PSUM",
                                                tag=f"mlp2_{ot}")
                    ps2.append(ps2_t)
                for k in range(ht_n):
                    for ot in range(ot_n):
                        nc.tensor.matmul(
                            out=ps2[ot][:],
                            lhsT=w2_sl(k, ot),
                            rhs=hid[k][:],
                            start=(k == 0), stop=(k == ht_n - 1))

                if gi == b_first:
                    ob_state["lo"] = lo
                    ob_state["hi"] = int(lay["grp_lo"][b_last])
                    strips = []
                    for o in range(ot_n):
                        ob_t = out_pool.tile(
                            [128, ob_state["hi"] - ob_state["lo"]],
                            BF16, tag=f"o{o}")
                        strips.append(ob_t)
                    ob_state["strips"] = strips
                ob_lo = ob_state["lo"]
                for ot in range(ot_n):
                    nc.vector.tensor_copy(
                        out=ob_state["strips"][ot][:, lo - ob_lo:
                                                   lo - ob_lo + w_g],
                        in_=ps2[ot][:])
                    if gi == b_last - 1:
                        q = nc.scalar if ot == 0 else nc.sync
                        q.dma_start(
                            out=out_p[ot, :, ob_lo:
                                      ob_lo + ob_state["strips"][ot].shape[1]],
                            in_=ob_state["strips"][ot][:])

            pend = None
            for gi in range(ngrp):
                lo = int(lay["grp_lo"][gi])
                w_g = int(lay["grp_lo"][gi + 1]) - lo
                nch = lay["nchunks"][gi]
                acols = lay["a_cols_g"][gi][-1]
                mb = lay["m_base"][gi]

                # ---- merged [A | stream] DMA for this group (SP queue)
                st = stream_pool.tile([128, acols + nch * 512], FP8, tag="hs")
                nc.sync.dma_start(
                    out=st[:], in_=hs_p[:, mb:mb + acols + nch * 512])

                # ---- rT strip (fp8, both ptiles in one DMA), RT_BATCH groups
                if gi % RT_BATCH == 0:
                    b_lo = lo
                    b_hi = int(lay["grp_lo"][min(gi + RT_BATCH, ngrp)])
                    b_w = b_hi - b_lo
                    rb_strip = r_pool.tile([128, fp * b_w], FP8, tag="rb")
                    nc.gpsimd.dma_start(
                        out=rb_strip[:],
                        in_=rT_p[:, fp * b_lo:fp * b_lo + fp * b_w])
                    rb_base = b_lo
                rb = [rb_strip[:, p * b_w + lo - rb_base:
                               p * b_w + lo - rb_base + w_g]
                      for p in range(fp)]

                # ---- segment-sum: per chunk, 4 matmuls (ptile x parity)
                # contract 128 edges at a time against the 0/1 A matrix
                ps_msg = []
                for p in range(fp):
                    mp_t = msg_psum_pool.tile([128, w_g], F32, space="PSUM",
                                              tag=f"mp{p}")
                    ps_msg.append(mp_t)
                offs = lay["a_cols_g"][gi]
                spans = lay["spans"][gi]
                for ch in range(nch):
                    p_lo, p_hi = spans[ch]
                    Wc = p_hi - p_lo
                    for parity in range(2):
                        a_sl = st[:, offs[ch] + parity * Wc:
                                  offs[ch] + (parity + 1) * Wc]
                        for p in range(fp):
                            c0 = acols + ch * 512 + parity * Fdim + p * 128
                            nc.tensor.matmul(
                                out=ps_msg[p][:, p_lo:p_hi],
                                lhsT=st[:, c0:c0 + 128],
                                rhs=a_sl,
                                start=(ch == 0 and parity == 0),
                                stop=(ch == nch - 1 and parity == 1),
                                skip_group_check=True,
                            )
                msgb = []
                for p in range(fp):
                    mb_t = msg_pool.tile([128, w_g], BF16, tag=f"mb{p}")
                    if p == 0:
                        nc.vector.tensor_copy(out=mb_t[:], in_=ps_msg[p][:])
                    else:
                        nc.scalar.activation(
                            mb_t[:], ps_msg[p][:],
                            mybir.ActivationFunctionType.Copy)
                    msgb.append(mb_t)

                # software pipeline: MLP of the previous group is emitted
                # AFTER this group's segment-sum so the PE never idles
                # waiting for message copies; the last two groups drop to
                # depth 0 so their MLPs overlap the tail streams
                emit_mlp({"gi": gi, "lo": lo, "w_g": w_g, "rb": rb,
                          "msgb": msgb})

    nc.finalize()
    return nc


# ----------------------------------------------------------------- interface
def prepare(r, h, nbrs, W1, W2):
    r = np.asarray(r, dtype=np.float32)
    h = np.asarray(h, dtype=np.float32)
    nbrs = np.asarray(nbrs)
    W1 = np.asarray(W1, dtype=np.float32)
    W2 = np.asarray(W2, dtype=np.float32)

    n_nodes, Fdim = r.shape
    H = W1.shape[1]
    Fout = W2.shape[1]

    dst = nbrs[:, 0].astype(np.int64)
    hq, deg, order, starts = _quantize_h(h, dst, n_nodes)
    lay = _preprocess(deg)
    hs_l, rT_l = _build_streams(hq, r, lay, order, starts)
    kt_n = (2 * Fdim) // 128
    ht_n = H // 128
    wb = np.zeros((128, kt_n * H + ht_n * Fout), dtype=NP_BF16)
    for k in range(kt_n):
        wb[:, k * H:(k + 1) * H] = W1[k * 128:(k + 1) * 128, :].astype(NP_BF16)
    for k in range(ht_n):
        wb[:, kt_n * H + k * Fout:kt_n * H + (k + 1) * Fout] = \
            W2[k * 128:(k + 1) * 128, :].astype(NP_BF16)

    nc = _build_graph(lay, Fdim, H, Fout)
    in_maps = [
        {"hs": hs_l[c], "rT": rT_l[c], "Wb": wb}
        for c in range(NC)
    ]
    return {"nc": nc, "in_maps": in_maps, "lay": lay,
            "n_nodes": n_nodes, "Fout": Fout}


def assemble(prep, results):
    lay = prep["lay"]
    n_nodes, Fout = prep["n_nodes"], prep["Fout"]
    npc = lay["npc"]
    out = np.zeros((n_nodes, Fout), dtype=np.float32)
    for c in range(NC):
        o = np.asarray(results[c]["out"]).reshape(Fout, npc)
        out[lay["node_ids"][c]] = o.T.astype(np.float32)
    return out


def kernel(r, h, nbrs, W1, W2):
    prep = prepare(r, h, nbrs, W1, W2)
    res = run_bass_kernel_spmd(prep["nc"], prep["in_maps"],
                               core_ids=list(range(NC)))
    return assemble(prep, res.results)


# revision 56
# speedup vs baseline: 1.0770x; 1.0100x over previous
"""ChemProp message-to-node + MLP kernel for 8 TRN2 NeuronCores.

Strategy (no collectives needed):
  - Host assigns nodes to cores by global degree rank, round-robin, so all
    cores see near-identical degree sequences.  Within a core, nodes are
    dealt round-robin into <=512-node groups (one PSUM window each).
  - Edge features stream in fp8 (e3m4) with host-side error-feedback
    quantization: each node's edges are rounded so quantization residuals
    carry into the next edge, making the on-device segment-sum nearly
    exact despite the 1-byte stream.
  - Segment-sum as true matmuls: edges are packed two-per-partition
    (512 B contiguous per partition keeps DMA at full bandwidth).  For
    each 256-edge chunk, 4 small matmuls (2 feature ptiles x even/odd
    slot) contract 128 edges at a time against a 0/1 aggregation matrix
    A streamed from HBM, accumulating the group's [128, w] message tile
    in PSUM.  Chunk boundaries are shared across cores via a max-degree
    ("ub") slot layout, so one Bass graph serves all 8 cores.
  - MLP runs in bf16 with f32 PSUM accumulation, feature-major, exactly
    as the reference: hidden^T = relu(W1^T @ [r; msg]^T), out = W2^T @ h.
  - Per-core output is returned feature-major bf16; host transposes,
    un-permutes, casts to f32 and concatenates.
"""

import numpy as np
import ml_dtypes

import concourse.bacc as bacc
import concourse.mybir as mybir
import concourse.tile as tile
from concourse.bass_utils import run_bass_kernel_spmd

NC = 8            # cores
# group caps: small first group -> fast pipeline fill; small last groups ->
# short drain; middle groups fill one PSUM f32 bank (512 cols)
CAPS = (256,) + (512,) * 10 + (352, 234, 162, 126)
SLOTS = 256       # edge slots per chunk (2 per partition)
STREAM_BUFS = 3
MSG_BUFS = 2
PSUM_MSG_BUFS = 2
HID_BUFS = 2
RT_BATCH = 4      # groups per rT load strip
OUT_BATCH = 2     # groups per out store strip

BF16 = mybir.dt.bfloat16
F32 = mybir.dt.float32
FP8 = mybir.dt.float8e3
NP_BF16 = ml_dtypes.bfloat16
NP_FP8 = ml_dtypes.float8_e3m4


# ----------------------------------------------------------------- host side
def _quantize_h(h, dst, n_nodes):
    """fp8 e3m4 with per-node error feedback: quantization residual of each
    edge is carried into the node's next edge, so the device-side sum of the
    quantized values tracks the exact sum to ~half an ulp."""
    deg = np.bincount(dst, minlength=n_nodes)
    order = np.argsort(dst, kind="stable")
    starts = np.zeros(n_nodes + 1, dtype=np.int64)
    np.cumsum(deg, out=starts[1:])
    hq = np.zeros(h.shape, dtype=NP_FP8)
    carry = np.zeros((n_nodes, h.shape[1]), dtype=np.float32)
    for k in range(int(deg.max())):
        sel = np.nonzero(deg > k)[0]
        eids = order[starts[sel] + k]
        val = h[eids].astype(np.float32) + carry[sel]
        q = val.astype(NP_FP8)
        carry[sel] = val - q.astype(np.float32)
        hq[eids] = q
    return hq, deg, order, starts


def _preprocess(deg_flat):
    """Node->core/group assignment + shared chunk schedule."""
    n_nodes = deg_flat.shape[0]
    npc = n_nodes // NC
    caps = list(CAPS)
    assert sum(caps) == npc and max(caps) <= 512
    grp_lo = np.concatenate([[0], np.cumsum(caps)]).astype(np.int64)
    ngrp = len(caps)

    # global degree rank, round-robin over cores, then round-robin over
    # groups within the core (fill order = degree-desc within group)
    rank = np.argsort(-deg_flat, kind="stable")
    node_ids = np.zeros((NC, npc), dtype=np.int64)
    for c in range(NC):
        ids_q = rank[c::NC]
        fill = [0] * ngrp
        g = 0
        for q in range(npc):
            while fill[g] == caps[g]:
                g = (g + 1) % ngrp
            node_ids[c, int(grp_lo[g]) + fill[g]] = ids_q[q]
            fill[g] += 1
            g = (g + 1) % ngrp
    deg_sorted = deg_flat[node_ids]                     # [NC, npc]

    # shared slot layout: node at (g, i) owns slots [cum_ub[i], cum_ub[i+1])
    # of group g, where deg_ub = max degree over cores at that position.
    deg_ub = deg_sorted.max(axis=0)                     # [npc]
    chunk_base = [0] * ngrp            # first global chunk of each group
    nchunks = [0] * ngrp
    spans = [None] * ngrp              # per chunk: (pos_lo, pos_hi)
    cum_ub_g = [None] * ngrp
    tot_chunks = 0
    for g in range(ngrp):
        lo, hi = int(grp_lo[g]), int(grp_lo[g + 1])
        ub = deg_ub[lo:hi]
        cum = np.zeros(hi - lo + 1, dtype=np.int64)
        np.cumsum(ub, out=cum[1:])
        cum_ub_g[g] = cum
        nch = max(int(-(-cum[-1] // SLOTS)), 1)
        chunk_base[g] = tot_chunks
        nchunks[g] = nch
        tot_chunks += nch
        sp = []
        for ch in range(nch):
            s0, s1 = ch * SLOTS, (ch + 1) * SLOTS
            # nodes whose slot window intersects [s0, s1); zero-degree nodes
            # fall into the chunk their cum position lands in
            p_lo = int(np.searchsorted(cum[1:], s0, side="right"))
            p_hi = int(np.searchsorted(cum[:-1], s1, side="left"))
            p_hi = max(p_hi, p_lo + 1)
            sp.append((p_lo, min(p_hi, hi - lo)))
        # every position must be covered by >=1 span so its PSUM message
        # column is written (zero-degree tail nodes otherwise fall through)
        sp[0] = (0, sp[0][1])
        sp[-1] = (sp[-1][0], hi - lo)
        for i in range(len(sp) - 1):
            assert sp[i + 1][0] <= sp[i][1]
        spans[g] = sp

    # A-matrix column layout: per group, per chunk: [even: W][odd: W]
    a_base = [0] * ngrp
    a_cols_g = [None] * ngrp
    a_tot = 0
    for g in range(ngrp):
        offs = []
        off = 0
        for (p_lo, p_hi) in spans[g]:
            offs.append(off)
            off += 2 * (p_hi - p_lo)
        a_base[g] = a_tot
        a_cols_g[g] = offs + [off]
        a_tot += off

    # merged per-group DMA block: [A acols | stream nch*512] fp8
    # (512 = 2 edges x 256 features per partition per chunk)
    m_base = [0] * ngrp
    m_tot = 0
    for g in range(ngrp):
        m_base[g] = m_tot
        m_tot += a_cols_g[g][-1] + nchunks[g] * 512

    return {
        "npc": npc, "ngrp": ngrp, "caps": caps, "grp_lo": grp_lo,
        "node_ids": node_ids, "deg_sorted": deg_sorted,
        "cum_ub_g": cum_ub_g, "chunk_base": chunk_base, "nchunks": nchunks,
        "spans": spans, "tot_chunks": tot_chunks,
        "a_base": a_base, "a_cols_g": a_cols_g, "a_tot": a_tot,
        "m_base": m_base, "m_tot": m_tot,
    }


def _build_streams(hq, r, lay, order, starts):
    """Materialize per-core device arrays: edge stream, A matrices, rT."""
    Fdim = hq.shape[1]
    npc, ngrp = lay["npc"], lay["ngrp"]
    tot_chunks, a_tot = lay["tot_chunks"], lay["a_tot"]
    grp_lo = lay["grp_lo"]
    fp = Fdim // 128

    hs_l, A_l, rT_l = [], [], []
    for c in range(NC):
        # edge -> slot assignment (vectorized per group)
        stream = np.zeros((tot_chunks * 128 * 2, Fdim), dtype=NP_FP8)
        A = np.zeros((128, a_tot), dtype=NP_FP8)
        for g in range(ngrp):
            lo, hi = int(grp_lo[g]), int(grp_lo[g + 1])
            nodes = lay["node_ids"][c, lo:hi]
            degs = lay["deg_sorted"][c, lo:hi]
            cum = lay["cum_ub_g"][g]
            E = int(degs.sum())
            if E == 0:
                continue
            loc = np.arange(E, dtype=np.int64) - np.repeat(
                np.concatenate([[0], np.cumsum(degs)[:-1]]), degs)
            slots = np.repeat(cum[:-1], degs) + loc        # slot in group
            eids = order[np.repeat(starts[nodes], degs) + loc]
            pos = np.repeat(np.arange(hi - lo, dtype=np.int64), degs)

            ch = slots // SLOTS
            within = slots % SLOTS
            part = within // 2
            parity = within % 2
            # stream row index: ((chunk_global*128 + part)*2 + parity)
            row = ((lay["chunk_base"][g] + ch) * 128 + part) * 2 + parity
            stream[row] = hq[eids]
            # A entry: col = a_base + chunk_off + parity*W + (pos - p_lo)
            offs = np.asarray(lay["a_cols_g"][g][:-1], dtype=np.int64)
            p_los = np.asarray([s[0] for s in lay["spans"][g]], dtype=np.int64)
            p_his = np.asarray([s[1] for s in lay["spans"][g]], dtype=np.int64)
            W = p_his - p_los
            col = lay["a_base"][g] + offs[ch] + parity * W[ch] + (pos - p_los[ch])
            assert (pos >= p_los[ch]).all() and (pos < p_his[ch]).all()
            A[part, col] = np.float32(1.0)

        # [tot_chunks, 128, 2, F] -> [128, tot_chunks * 2F] partition-major,
        # then interleave each group's A block in front of its stream block
        st = stream.reshape(tot_chunks, 128, 2 * Fdim)
        sm = np.ascontiguousarray(st.transpose(1, 0, 2)).reshape(128, -1)
        merged = np.zeros((128, lay["m_tot"]), dtype=NP_FP8)
        for g in range(ngrp):
            mb = lay["m_base"][g]
            ab = lay["a_base"][g]
            acols = lay["a_cols_g"][g][-1]
            cb = lay["chunk_base"][g]
            nch = lay["nchunks"][g]
            merged[:, mb:mb + acols] = A[:, ab:ab + acols]
            merged[:, mb + acols:mb + acols + nch * 512] = \
                sm[:, cb * 512:(cb + nch) * 512]
        hs_l.append(merged)
        # rT strips: per RT_BATCH block, both feature ptiles side by side so
        # one DMA fetches the whole strip: cols [2*b_lo:2*b_hi] =
        # [p0 cols b_lo..b_hi | p1 cols b_lo..b_hi]
        rc = r[lay["node_ids"][c]].astype(NP_FP8)
        rt = np.ascontiguousarray(rc.T).reshape(fp, 128, npc)
        rti = np.zeros((128, fp * npc), dtype=NP_FP8)
        bs = [int(x) for x in lay["grp_lo"][::RT_BATCH]] + [npc]
        for b_lo, b_hi in zip(bs[:-1], bs[1:]):
            w = b_hi - b_lo
            for p in range(fp):
                rti[:, fp * b_lo + p * w:fp * b_lo + (p + 1) * w] = \
                    rt[p, :, b_lo:b_hi]
        rT_l.append(rti)
    return hs_l, rT_l


# --------------------------------------------------------------- device side
def _build_graph(lay, Fdim, H, Fout):
    npc, ngrp = lay["npc"], lay["ngrp"]
    fp = Fdim // 128          # 2 feature ptiles
    kt_n = (2 * Fdim) // 128  # 4 k-chunks for W1
    ht_n = H // 128           # 4 hidden ptiles
    ot_n = Fout // 128        # 2 output ptiles

    nc = bacc.Bacc(None, target_bir_lowering=False)
    hs_p = nc.declare_dram_parameter("hs", [128, lay["m_tot"]], FP8, isOutput=False)
    rT_p = nc.declare_dram_parameter("rT", [128, fp * npc], FP8, isOutput=False)
    # W1 and W2 packed partition-major into one blob -> single DMA issue
    wb_cols = kt_n * H + ht_n * Fout
    wb_p = nc.declare_dram_parameter("Wb", [128, wb_cols], BF16, isOutput=False)
    out_p = nc.declare_dram_parameter("out", [ot_n, 128, npc], BF16, isOutput=True)

    with tile.TileContext(nc) as tc:
        with (
            tc.tile_pool(name="const", bufs=1) as const_pool,
            tc.tile_pool(name="stream", bufs=STREAM_BUFS) as stream_pool,
            tc.tile_pool(name="msgp", bufs=1, space="PSUM") as msg_psum_pool,
            tc.tile_pool(name="msgb", bufs=MSG_BUFS) as msg_pool,
            tc.tile_pool(name="rb", bufs=2) as r_pool,
            tc.tile_pool(name="mlp1p", bufs=3, space="PSUM") as mlp1_psum_pool,
            tc.tile_pool(name="mlp2p", bufs=1, space="PSUM") as mlp2_psum_pool,
            tc.tile_pool(name="hid", bufs=HID_BUFS) as hid_pool,
            tc.tile_pool(name="osb", bufs=3) as out_pool,
        ):
            # weights resident in SBUF; one DMA on the SP queue ahead of the
            # first edge-stream DMA so MLP(g0) never stalls on them
            wb_t = const_pool.tile([128, wb_cols], BF16, tag="wb")
            nc.sync.dma_start(out=wb_t[:], in_=wb_p[:])

            def w1_sl(k, ht):
                c = k * H + ht * 128
                return wb_t[:, c:c + 128]

            def w2_sl(k, ot):
                c = kt_n * H + k * Fout + ot * 128
                return wb_t[:, c:c + 128]

            # out-store batches: OUT_BATCH groups each, but the last two
            # groups flush individually so the final store doesn't wait on
            # two MLPs
            batch_of = {}
            batches = []
            g = 0
            while g < ngrp:
                n = 1 if g >= ngrp - 2 else min(OUT_BATCH, ngrp - 2 - g)
                batches.append((g, g + n))
                for x in range(g, g + n):
                    batch_of[x] = len(batches) - 1
                g += n
            ob_state = {}

            def emit_mlp(pend):
                gi, lo, w_g = pend["gi"], pend["lo"], pend["w_g"]
                cat = pend["rb"] + pend["msgb"]  # k-chunks match W1 rows
                b_first, b_last = batches[batch_of[gi]]
                hid = []
                for ht in range(ht_n):
                    ps = mlp1_psum_pool.tile([128, w_g], F32, space="PSUM",
                                             tag="mlp1")
                    for k in range(kt_n):
                        nc.tensor.matmul(
                            out=ps[:],
                            lhsT=w1_sl(k, ht),
                            rhs=cat[k][:],
                            start=(k == 0), stop=(k == kt_n - 1))
                    hb = hid_pool.tile([128, w_g], BF16, tag=f"h{ht}")
                    if ht % 2 == 0:
                        nc.scalar.activation(
                            hb[:], ps[:], mybir.ActivationFunctionType.Relu)
                    else:
                        nc.vector.tensor_scalar_max(hb[:], ps[:], 0.0)
                    hid.append(hb)
                # k-major so the last-relu'd hidden tile is consumed LAST,
                # with both output tiles' earlier k-chunks runnable before it
                ps2 = []
                for ot in range(ot_n):
                    ps2_t = mlp2_psum_pool.tile([128, w_g], F32, space="PSUM",
                                                tag=f"mlp2_{ot}")
                    ps2.append(ps2_t)
                for k in range(ht_n):
                    for ot in range(ot_n):
                        nc.tensor.matmul(
                            out=ps2[ot][:],
                            lhsT=w2_sl(k, ot),
                            rhs=hid[k][:],
                            start=(k == 0), stop=(k == ht_n - 1))

                if gi == b_first:
                    ob_state["lo"] = lo
                    ob_state["hi"] = int(lay["grp_lo"][b_last])
                    strips = []
                    for o in range(ot_n):
                        ob_t = out_pool.tile(
                            [128, ob_state["hi"] - ob_state["lo"]],
                            BF16, tag=f"o{o}")
                        strips.append(ob_t)
                    ob_state["strips"] = strips
                ob_lo = ob_state["lo"]
                for ot in range(ot_n):
                    nc.vector.tensor_copy(
                        out=ob_state["strips"][ot][:, lo - ob_lo:
                                                   lo - ob_lo + w_g],
                        in_=ps2[ot][:])
                    if gi == b_last - 1:
                        q = nc.scalar if ot == 0 else nc.sync
                        q.dma_start(
                            out=out_p[ot, :, ob_lo:
                                      ob_lo + ob_state["strips"][ot].shape[1]],
                            in_=ob_state["strips"][ot][:])

            pend = None
            for gi in range(ngrp):
                lo = int(lay["grp_lo"][gi])
                w_g = int(lay["grp_lo"][gi + 1]) - lo
                nch = lay["nchunks"][gi]
                acols = lay["a_cols_g"][gi][-1]
                mb = lay["m_base"][gi]

                # ---- merged [A | stream] DMA for this group (SP queue)
                st = stream_pool.tile([128, acols + nch * 512], FP8, tag="hs")
                nc.sync.dma_start(
                    out=st[:], in_=hs_p[:, mb:mb + acols + nch * 512])

                # ---- rT strip (fp8, both ptiles in one DMA), RT_BATCH groups
                if gi % RT_BATCH == 0:
                    b_lo = lo
                    b_hi = int(lay["grp_lo"][min(gi + RT_BATCH, ngrp)])
                    b_w = b_hi - b_lo
                    rb_strip = r_pool.tile([128, fp * b_w], FP8, tag="rb")
                    nc.gpsimd.dma_start(
                        out=rb_strip[:],
                        in_=rT_p[:, fp * b_lo:fp * b_lo + fp * b_w])
                    rb_base = b_lo
                rb = [rb_strip[:, p * b_w + lo - rb_base:
                               p * b_w + lo - rb_base + w_g]
                      for p in range(fp)]

                # ---- segment-sum: per chunk, 4 matmuls (ptile x parity)
                # contract 128 edges at a time against the 0/1 A matrix
                ps_msg = []
                for p in range(fp):
                    mp_t = msg_psum_pool.tile([128, w_g], F32, space="PSUM",
                                              tag=f"mp{p}")
                    ps_msg.append(mp_t)
                offs = lay["a_cols_g"][gi]
                spans = lay["spans"][gi]
                for ch in range(nch):
                    p_lo, p_hi = spans[ch]
                    Wc = p_hi - p_lo
                    for parity in range(2):
                        a_sl = st[:, offs[ch] + parity * Wc:
                                  offs[ch] + (parity + 1) * Wc]
                        for p in range(fp):
                            c0 = acols + ch * 512 + parity * Fdim + p * 128
                            nc.tensor.matmul(
                                out=ps_msg[p][:, p_lo:p_hi],
                                lhsT=st[:, c0:c0 + 128],
                                rhs=a_sl,
                                start=(ch == 0 and parity == 0),
                                stop=(ch == nch - 1 and parity == 1),
                                skip_group_check=True,
                            )
                msgb = []
                for p in range(fp):
                    mb_t = msg_pool.tile([128, w_g], BF16, tag=f"mb{p}")
                    if p == 0:
                        nc.vector.tensor_copy(out=mb_t[:], in_=ps_msg[p][:])
                    else:
                        nc.scalar.activation(
                            mb_t[:], ps_msg[p][:],
                            mybir.ActivationFunctionType.Copy)
                    msgb.append(mb_t)

                # software pipeline: MLP of the previous group is emitted
                # AFTER this group's segment-sum so the PE never idles
                # waiting for message copies; the last two groups drop to
                # depth 0 so their MLPs overlap the tail streams
                # depth-1 software pipeline: MLP(g-1) is emitted after
                # seg(g), hiding msg-copy latency behind the previous MLP;
                # the final group drops to depth 0 to shorten the drain
                cur = {"gi": gi, "lo": lo, "w_g": w_g, "rb": rb,
                       "msgb": msgb}
                if pend is not None:
                    emit_mlp(pend)
                    pend = None
                if gi == ngrp - 1:
                    emit_mlp(cur)
                else:
                    pend = cur

    nc.finalize()
    return nc


# ----------------------------------------------------------------- interface
def prepare(r, h, nbrs, W1, W2):
    r = np.asarray(r, dtype=np.float32)
    h = np.asarray(h, dtype=np.float32)
    nbrs = np.asarray(nbrs)
    W1 = np.asarray(W1, dtype=np.float32)
    W2 = np.asarray(W2, dtype=np.float32)

    n_nodes, Fdim = r.shape
    H = W1.shape[1]
    Fout = W2.shape[1]

    dst = nbrs[:, 0].astype(np.int64)
    hq, deg, order, starts = _quantize_h(h, dst, n_nodes)
    lay = _preprocess(deg)
    hs_l, rT_l = _build_streams(hq, r, lay, order, starts)
    kt_n = (2 * Fdim) // 128
    ht_n = H // 128
    wb = np.zeros((128, kt_n * H + ht_n * Fout), dtype=NP_BF16)
    for k in range(kt_n):
        wb[:, k * H:(k + 1) * H] = W1[k * 128:(k + 1) * 128, :].astype(NP_BF16)
    for k in range(ht_n):
        wb[:, kt_n * H + k * Fout:kt_n * H + (k + 1) * Fout] = \
            W2[k * 128:(k + 1) * 128, :].astype(NP_BF16)

    nc = _build_graph(lay, Fdim, H, Fout)
    in_maps = [
        {"hs": hs_l[c], "rT": rT_l[c], "Wb": wb}
        for c in range(NC)
    ]
    return {"nc": nc, "in_maps": in_maps, "lay": lay,
            "n_nodes": n_nodes, "Fout": Fout}


def assemble(prep, results):
    lay = prep["lay"]
    n_nodes, Fout = prep["n_nodes"], prep["Fout"]
    npc = lay["npc"]
    out = np.zeros((n_nodes, Fout), dtype=np.float32)
    for c in range(NC):
        o = np.asarray(results[c]["out"]).reshape(Fout, npc)
        out[lay["node_ids"][c]] = o.T.astype(np.float32)
    return out


def kernel(r, h, nbrs, W1, W2):
    prep = prepare(r, h, nbrs, W1, W2)
    res = run_bass_kernel_spmd(prep["nc"], prep["in_maps"],
                               core_ids=list(range(NC)))
    return assemble(prep, res.results)
